# revision 45
# baseline (speedup 1.0000x reference)
"""MultiHeadAttnBlock TRN2 kernel (v2).

Full inputs -> shard across 8 NeuronCores -> full output.
Core i handles (batch b = i//4, spatial quarter sq = i%4): K/V over the
full spatial dim, Q over its quarter, 4-head attention for 1024 queries
x 4096 keys, wo projection, residual.

v2 changes vs the 235us baseline:
 - group-norm folded into the 1x1-conv weights: wkA/wvA = w * Ax[c],
   wqA = wq * Ay[c]; k-side biases vanish through softmax, the v-side
   bias is restored as wo@(wv@Bx) on the output, the q-side bias as a
   device matvec added at the q drain.  The [128,4096] normalize passes
   disappear.
 - scores for the two heads of a pair are emitted back-to-back into
   different PSUM banks with K=64 at partitions 0-63/64-127: the PE
   row-tiles them and streams both concurrently (~2x on scores).
 - one [128,2048] f32 score region per t-tile (banks 0-3), layout
   [h0n0|h1n0|h0n1|h1n1]; exp is issued per 2-bank half so the next
   tile's score matmuls ping-pong with the exp reads.
 - exp split across engines: ScalarE half0 (table exp), VectorE half1 on
   scheduled tiles via a bit-trick: i16 = rint(s*184.665+16250.49)
   reinterpreted as bf16 is exp(s) to ~3%; the denominator uses the same
   approximation so softmax cancels most of it.
 - rsqrt for the group stats via the 0x5f3759df bit trick + 2 Newton
   steps on VectorE: no Sqrt table set, single exp table load warmed at
   kernel start.
"""

import numpy as np
import ml_dtypes

import concourse.bass as bass
import concourse.mybir as mybir
import bass_rust as _br
from concourse.tile import TileContext
from concourse.bass_utils import run_bass_kernel_spmd

F32 = mybir.dt.float32
BF16 = mybir.dt.bfloat16
I16 = mybir.dt.int16
I32 = mybir.dt.int32
AF = mybir.ActivationFunctionType
OP = mybir.AluOpType

C = 256          # channels
S = 4096         # spatial (64*64)
SQ = 1024        # spatial quarter handled per core
H = 4            # heads
D = 64           # head dim
G = 32           # groups
EPS = 1e-6
NT = 32          # t tiles of 128 over S
VW = D + 2       # v' width per head (v | ones | pad)

EXPS = 184.66496523378732      # 128*log2(e)
EXPB = 16250.4931              # 128*127 - minimax centering
QUAKE = 1597463007.0           # 0x5f3759df


def _split_sched(p):
    """True -> VectorE computes the half1 exp of this t-tile."""
    if p == 0:
        return [t % 4 != 3 for t in range(NT)]
    return [t % 8 != 7 for t in range(NT)]


def build_nc():
    nc = bass.Bass("TRN2", target_bir_lowering=False, debug=False, num_devices=8)

    def din(name, shape, dt=F32):
        return nc.dram_tensor(name, shape, dt, kind="ExternalInput").ap()

    x_d = din("x", [C, S], BF16)    # full batch slice (stats + k/v)
    y_d = din("y", [C, S], BF16)    # full batch slice (stats + q quarter)
    xq_d = din("xq", [C, SQ])       # residual quarter + bo2, f32
    wqT_d = din("wqT", [C, C], BF16)   # wq.T / 8
    wkT_d = din("wkT", [C, C], BF16)
    wvT_d = din("wvT", [C, C], BF16)
    woT_d = din("woT", [C, C], BF16)
    # packed per-channel vectors: cols = (bq8, bo2, g1, b1, g2, b2)
    vecs_d = din("vecs", [C, 6])
    pool_d = din("poolm", [C, G])   # (c//8==g)/8
    exp_d = din("expandm", [G, C])  # (c//8==g)
    out_d = nc.dram_tensor("out", [C, SQ], F32, kind="ExternalOutput").ap()
    rcd = [nc.dram_tensor(f"rcd{i}", [1, SQ], BF16).ap() for i in range(2)]

    with TileContext(nc) as tc:
        with (
            tc.tile_pool(name="pers", bufs=1) as pers,
            tc.tile_pool(name="sb1", bufs=1) as sb1,
            tc.tile_pool(name="sb2", bufs=2) as sb2,
            tc.tile_pool(name="expp", bufs=3) as expp,
            tc.tile_pool(name="ps", bufs=1, space="PSUM") as ps,
        ):
            # ---- persistent tiles -------------------------------------
            xf = [pers.tile([128, S], BF16, tag=f"xf{m}", name=f"xf{m}")
                  for m in range(2)]
            yf = [pers.tile([128, S], BF16, tag=f"yf{m}", name=f"yf{m}")
                  for m in range(2)]
            xq = [pers.tile([128, SQ], F32, tag=f"xq{m}", name=f"xq{m}")
                  for m in range(2)]
            k_sb = [[pers.tile([128, 1024], BF16, tag=f"ksb{m}_{j}",
                               name=f"ksb{m}_{j}") for j in range(4)]
                    for m in range(2)]
            q_sb = [pers.tile([128, SQ], BF16, tag=f"qsb{m}", name=f"qsb{m}")
                    for m in range(2)]
            v_sb = [pers.tile([128, 8 * H * VW], BF16, tag=f"vsb{j}",
                              name=f"vsb{j}") for j in range(4)]
            out_ds = [pers.tile([128, SQ], BF16, tag=f"ods{m}", name=f"ods{m}")
                      for m in range(2)]
            wq_b = [pers.tile([128, C], BF16, tag=f"wqb{m}", name=f"wqb{m}")
                    for m in range(2)]
            wk_b = [pers.tile([128, C], BF16, tag=f"wkb{m}", name=f"wkb{m}")
                    for m in range(2)]
            wv_b = [pers.tile([128, C], BF16, tag=f"wvb{m}", name=f"wvb{m}")
                    for m in range(2)]
            wo_b = [pers.tile([128, C], BF16, tag=f"wob{m}", name=f"wob{m}")
                    for m in range(2)]
            wqA = [pers.tile([128, C], BF16, tag=f"wqA{m}", name=f"wqA{m}")
                   for m in range(2)]
            wkA = [pers.tile([128, C], BF16, tag=f"wkA{m}", name=f"wkA{m}")
                   for m in range(2)]
            wvA = [pers.tile([128, C], BF16, tag=f"wvA{m}", name=f"wvA{m}")
                   for m in range(2)]
            vecs = [pers.tile([128, 6], F32, tag=f"vecs{m}", name=f"vecs{m}")
                    for m in range(2)]
            _vc = {"bq8": 0, "bo2": 1, "g1": 2, "b1": 3, "g2": 4, "b2": 5}
            gb = {nm: [vecs[m][:, i:i + 1] for m in range(2)]
                  for nm, i in _vc.items()}
            cq8 = [pers.tile([128, 1], F32, tag=f"cq8{m}", name=f"cq8{m}")
                   for m in range(2)]
            cv_sb = [pers.tile([128, 1], F32, tag=f"cv{m}", name=f"cv{m}")
                     for m in range(2)]
            wocv = [pers.tile([128, 1], F32, tag=f"wocv{m}", name=f"wocv{m}")
                    for m in range(2)]
            den32 = pers.tile([32, 64], F32, tag="den32", name="den32")
            rc32 = pers.tile([32, 64], BF16, tag="rc32", name="rc32")
            warm = pers.tile([128, 2], F32, tag="warm", name="warm")

            # ones column (64) + pad (65) of each v' head block
            for j in range(4):
                vview = v_sb[j][:].rearrange("p (t h e) -> p t h e", t=8, h=H)
                nc.gpsimd.memset(vview[:, :, :, D:VW], 1.0)

            # ---- stage 1: inputs + group-norm stats --------------------
            s6x = [sb1.tile([128, 24], F32, tag=f"s6x{m}", name=f"s6x{m}")
                   for m in range(2)]
            s6y = [sb1.tile([128, 24], F32, tag=f"s6y{m}", name=f"s6y{m}")
                   for m in range(2)]

            # tiny constants first: they gate the affine matmuls and must
            # not queue behind the big x/y transfers
            pool_sb = [sb1.tile([128, G], F32, tag=f"pl{m}", name=f"pl{m}")
                       for m in range(2)]
            expand_sb = sb1.tile([G, C], F32, tag="ex", name="ex")
            for m in range(2):
                nc.sync.dma_start(out=pool_sb[m][:],
                                  in_=pool_d[m * 128:(m + 1) * 128, :])
                nc.sync.dma_start(out=vecs[m][:],
                                  in_=vecs_d[m * 128:(m + 1) * 128, :])
            nc.sync.dma_start(out=expand_sb[:], in_=exp_d[:])
            # exp table warm-up: load the set while DMAs stream
            nc.scalar.activation(warm[:], vecs[0][:, 0:2], AF.Exp)

            # x first (k/v gate the pipeline): 8 chunks per half; stats on
            # alternating 512-chunks (half the DVE time, ~0.3% stat noise);
            # stat chunks DMA'd first so the stats finish early
            for ch in (0, 2, 4, 6, 1, 3, 5, 7):
                for m in range(2):
                    cs = slice(m * 128, (m + 1) * 128)
                    fs = slice(ch * 512, (ch + 1) * 512)
                    if ch % 2 == 0:
                        # stat chunks split across two queues so the first
                        # bn_stats can start ~2us earlier
                        for h2 in range(2):
                            f2 = slice(ch * 512 + h2 * 256,
                                       ch * 512 + h2 * 256 + 256)
                            nc.sync.dma_start(out=xf[m][:, f2],
                                              in_=x_d[cs, f2])
                        c2 = ch // 2
                        nc.vector.bn_stats(s6x[m][:, c2 * 6:(c2 + 1) * 6],
                                           xf[m][:, fs])
                    else:
                        nc.sync.dma_start(out=xf[m][:, fs], in_=x_d[cs, fs])
            for m in range(2):
                nc.sync.dma_start(out=wk_b[m][:],
                                  in_=wkT_d[m * 128:(m + 1) * 128, :])
                nc.sync.dma_start(out=wv_b[m][:],
                                  in_=wvT_d[m * 128:(m + 1) * 128, :])
            for m in range(2):
                cs = slice(m * 128, (m + 1) * 128)
                for ch in (0, 2, 4, 6, 1, 3, 5, 7):
                    fs = slice(ch * 512, (ch + 1) * 512)
                    nc.sync.dma_start(out=yf[m][:, fs], in_=y_d[cs, fs])
            for m in range(2):
                cs = slice(m * 128, (m + 1) * 128)
                nc.sync.dma_start(out=wq_b[m][:], in_=wqT_d[cs, :])
                nc.sync.dma_start(out=wo_b[m][:], in_=woT_d[cs, :])
                for ch in range(2):
                    fs = slice(ch * 512, (ch + 1) * 512)
                    nc.sync.dma_start(out=xq[m][:, fs], in_=xq_d[cs, fs])

            def group_affine(s6, gamma, beta, tagp):
                """per-channel A, B [128,1] x2 from bn_stats chunks; rsqrt
                via the 0x5f3759df bit trick + 2 Newton steps (DVE only)"""
                ve = nc.vector
                stats_c = []
                for m in range(2):
                    mv = sb1.tile([128, 2], F32, tag=f"mv{tagp}{m}",
                                  name=f"mv{tagp}{m}")
                    ve.bn_aggr(mv[:], s6[m][:])
                    st = sb1.tile([128, 2], F32, tag=f"st{tagp}{m}",
                                  name=f"st{tagp}{m}")
                    ve.tensor_copy(st[:, 0:1], mv[:, 0:1])
                    msq = sb1.tile([128, 1], F32, tag=f"msq{tagp}{m}",
                                   name=f"msq{tagp}{m}")
                    ve.tensor_mul(msq[:], mv[:, 0:1], mv[:, 0:1])
                    ve.tensor_add(st[:, 1:2], mv[:, 1:2], msq[:])
                    stats_c.append(st)
                gp = ps.tile([G, 2], F32, tag="psD", padded_shape=[128, 1024],
                             name=f"gp{tagp}")
                for m in range(2):
                    nc.tensor.matmul(gp[:], lhsT=pool_sb[m][:],
                                     rhs=stats_c[m][:],
                                     start=(m == 0), stop=(m == 1))
                gs = sb1.tile([G, 2], F32, tag=f"gs{tagp}", name=f"gs{tagp}")
                nc.vector.tensor_copy(gs[:], gp[:])
                musq = sb1.tile([G, 1], F32, tag=f"gmusq{tagp}",
                                name=f"gmusq{tagp}")
                ve.tensor_mul(musq[:], gs[:, 0:1], gs[:, 0:1])
                veps = sb1.tile([G, 1], F32, tag=f"veps{tagp}",
                                name=f"veps{tagp}")
                ve.tensor_sub(veps[:], gs[:, 1:2], musq[:])
                ve.tensor_scalar_add(veps[:], veps[:], EPS)
                # quake rsqrt seed: float(bits) -> bits' = Q - bits/2
                fb = sb1.tile([G, 1], F32, tag=f"fb{tagp}", name=f"fb{tagp}")
                ve.tensor_copy(fb[:], veps[:].bitcast(I32))
                gi = sb1.tile([G, 1], I32, tag=f"gi{tagp}", name=f"gi{tagp}")
                ve.tensor_scalar(gi[:], fb[:], -0.5, QUAKE, OP.mult, OP.add)
                r = gi[:].bitcast(F32)
                # 2 Newton steps: r *= 1.5 - 0.5*veps*r^2
                y2 = sb1.tile([G, 1], F32, tag=f"gy2{tagp}",
                              name=f"gy2{tagp}")
                rr = sb1.tile([G, 1], F32, tag=f"grr{tagp}",
                              name=f"grr{tagp}")
                ve.tensor_mul(y2[:], r, r)
                ve.tensor_mul(y2[:], veps[:], y2[:])
                ve.tensor_scalar(y2[:], y2[:], -0.5, 1.5, OP.mult, OP.add)
                ve.tensor_mul(rr[:], r, y2[:])
                ve.tensor_mul(y2[:], rr[:], rr[:])
                ve.tensor_mul(y2[:], veps[:], y2[:])
                ve.tensor_scalar(y2[:], y2[:], -0.5, 1.5, OP.mult, OP.add)
                gs2 = sb1.tile([G, 2], F32, tag=f"gs2{tagp}",
                               name=f"gs2{tagp}")
                ve.tensor_mul(gs2[:, 0:1], rr[:], y2[:])
                ve.tensor_copy(gs2[:, 1:2], gs[:, 0:1])
                A, B = [], []
                for m in range(2):
                    pc = ps.tile([128, 2], F32, tag="psD",
                                 padded_shape=[128, 1024],
                                 name=f"pc{tagp}{m}")
                    nc.tensor.matmul(
                        pc[:], lhsT=expand_sb[:, m * 128:(m + 1) * 128],
                        rhs=gs2[:], start=True, stop=True)
                    a = sb1.tile([128, 1], F32, tag=f"A{tagp}{m}",
                                 name=f"A{tagp}{m}")
                    nc.vector.tensor_mul(a[:], pc[:, 0:1], gamma[m])
                    bmid = sb1.tile([128, 1], F32, tag=f"Bm{tagp}{m}",
                                    name=f"Bm{tagp}{m}")
                    nc.vector.tensor_mul(bmid[:], pc[:, 1:2], a[:])
                    b_ = sb1.tile([128, 1], F32, tag=f"B{tagp}{m}",
                                  name=f"B{tagp}{m}")
                    ve.tensor_sub(b_[:], beta[m], bmid[:])
                    A.append(a)
                    B.append(b_)
                return A, B

            Ax, Bx = group_affine(s6x, gb["g1"], gb["b1"], "x")
            for m in range(2):
                nc.vector.tensor_scalar(wkA[m][:], wk_b[m][:],
                                        Ax[m][:, 0:1], None, OP.mult)
                nc.vector.tensor_scalar(wvA[m][:], wv_b[m][:],
                                        Ax[m][:, 0:1], None, OP.mult)
            # cv = wv @ Bx (v bias from the folded norm; bv is host-folded)
            Bx16 = [sb1.tile([128, 1], BF16, tag=f"Bx16{m}", name=f"Bx16{m}")
                    for m in range(2)]
            cv16 = [sb1.tile([128, 1], BF16, tag=f"cv16{m}", name=f"cv16{m}")
                    for m in range(2)]
            for m in range(2):
                nc.vector.tensor_copy(Bx16[m][:], Bx[m][:])
            for m in range(2):
                pcv = ps.tile([128, 1], F32, tag="psC",
                              padded_shape=[128, 1024], name=f"pcv{m}")
                for kk in range(2):
                    nc.tensor.matmul(pcv[:],
                                     lhsT=wv_b[kk][:, m * 128:(m + 1) * 128],
                                     rhs=Bx16[kk][:, 0:1],
                                     start=(kk == 0), stop=(kk == 1))
                nc.vector.tensor_copy(cv16[m][:], pcv[:])

            # y stats AFTER the x affine chain: emission order is DVE
            # execution order, and affine-x gates the k/v projections
            for m in range(2):
                for c2 in range(4):
                    fs = slice(c2 * 1024, c2 * 1024 + 512)
                    nc.vector.bn_stats(s6y[m][:, c2 * 6:(c2 + 1) * 6],
                                       yf[m][:, fs])

            # ---- stage 2: k projection (no bias: softmax-invariant) ----
            for m in range(2):
                for n in range(0, S, 512):
                    pk = ps.tile([128, 512], F32,
                                 tag="psC" if (n // 512) % 2 == 0 else "psD",
                                 padded_shape=[128, 1024], name=f"pk{m}_{n}")
                    for kk in range(2):
                        nc.tensor.matmul(
                            pk[:],
                            lhsT=wkA[kk][:, m * 128:(m + 1) * 128],
                            rhs=xf[kk][:, n:n + 512],
                            start=(kk == 0), stop=(kk == 1))
                    kdst = k_sb[m][n // 1024][:, n % 1024:n % 1024 + 512]
                    nc.scalar.copy(kdst, pk[:])

            # ---- stage 2c: y affine, q projection ----------------------
            Ay, By = group_affine(s6y, gb["g2"], gb["b2"], "y")
            for m in range(2):
                nc.vector.tensor_scalar(wqA[m][:], wq_b[m][:],
                                        Ay[m][:, 0:1], None, OP.mult)
            # cq8 = (wq/8) @ By + bq/8
            By16 = [sb1.tile([128, 1], BF16, tag=f"By16{m}", name=f"By16{m}")
                    for m in range(2)]
            for m in range(2):
                nc.vector.tensor_copy(By16[m][:], By[m][:])
            for m in range(2):
                pcq = ps.tile([128, 1], F32, tag="psD",
                              padded_shape=[128, 1024], name=f"pcq{m}")
                for kk in range(2):
                    nc.tensor.matmul(pcq[:],
                                     lhsT=wq_b[kk][:, m * 128:(m + 1) * 128],
                                     rhs=By16[kk][:, 0:1],
                                     start=(kk == 0), stop=(kk == 1))
                nc.vector.tensor_scalar(cq8[m][:], pcq[:], gb["bq8"][m],
                                        None, OP.add)
            # wocv = wo @ cv (added at the output drain)
            for m in range(2):
                pwo = ps.tile([128, 1], F32, tag="psC",
                              padded_shape=[128, 1024], name=f"pwo{m}")
                for kk in range(2):
                    nc.tensor.matmul(pwo[:],
                                     lhsT=wo_b[kk][:, m * 128:(m + 1) * 128],
                                     rhs=cv16[kk][:, 0:1],
                                     start=(kk == 0), stop=(kk == 1))
                nc.vector.tensor_copy(wocv[m][:], pwo[:])
            # q projection over the quarter (host rolls y so it is cols 0:SQ)
            for m in range(2):
                pq = ps.tile([128, SQ], F32,
                             tag="psSA" if m == 0 else "psSB",
                             padded_shape=[128, 1024], name=f"pq{m}")
                for n in range(0, SQ, 512):
                    for kk in range(2):
                        nc.tensor.matmul(
                            pq[:, n:n + 512],
                            lhsT=wqA[kk][:, m * 128:(m + 1) * 128],
                            rhs=yf[kk][:, n:n + 512],
                            start=(kk == 0), stop=(kk == 1))
                nc.vector.tensor_scalar(q_sb[m][:], pq[:], cq8[m][:, 0:1],
                                        None, OP.add)

            # ---- stage 2b: v projection (all 32 t tiles) ---------------
            for t in range(NT):
                pv = ps.tile([128, C], F32,
                             tag="psC" if t % 2 == 0 else "psD",
                             padded_shape=[128, 1024], name=f"pv{t}")
                tsl = slice(t * 128, (t + 1) * 128)
                for kk in range(2):
                    nc.tensor.matmul(pv[:], lhsT=xf[kk][:, tsl],
                                     rhs=wvA[kk][:],
                                     start=(kk == 0), stop=(kk == 1))
                pvv = pv[:].rearrange("p (h e) -> p h e", h=H)
                dst = v_sb[t // 8][:, (t % 8) * H * VW:(t % 8 + 1) * H * VW]
                dvv = dst.rearrange("p (h e) -> p h e", h=H)[:, :, 0:D]
                if t < 24:
                    nc.scalar.copy(dvv, pvv)
                else:
                    nc.vector.tensor_copy(dvv, pvv)

            # ---- stage 3: attention ------------------------------------
            po = []
            for p in range(2):
                # two separate score tiles: Tile serializes cross-engine
                # accesses to one PSUM tile, so the ACT-read half and the
                # DVE-read half must be distinct tiles to run concurrently
                scA = ps.tile([128, 1024], F32, tag="psSA", name=f"scA{p}")
                scB = ps.tile([128, 1024], F32, tag="psSB", name=f"scB{p}")
                acc = [ps.tile([VW, SQ], F32, tag=["psC", "psD"][hh],
                               padded_shape=[128, 1024],
                               name=f"acc{p}_{hh}") for hh in range(2)]
                split = _split_sched(p)

                def emit_scores(t, nsel=(0, 512)):
                    # scA = [h0n0 | h1n0], scB = [h0n1 | h1n1]; head pair
                    # back-to-back -> concurrent row-tiled matmuls
                    tsl = slice((t % 8) * 128, (t % 8 + 1) * 128)
                    kt = k_sb[p][t // 8]
                    for n in nsel:
                        sct = scA if n == 0 else scB
                        for hh in range(2):
                            lo = hh * 64
                            nc.tensor.matmul(
                                sct[:, hh * 512:hh * 512 + 512],
                                lhsT=kt[lo:lo + 64, tsl],
                                rhs=q_sb[p][lo:lo + 64, n:n + 512],
                                start=True, stop=True)

                emit_scores(0)
                for t in range(NT):
                    # separate tiles per half so the ACT and DVE exps are
                    # independent writes and run concurrently
                    e0 = expp.tile([128, 1024], BF16, tag="e0",
                                   name=f"e0_{p}_{t}")
                    e1 = expp.tile([128, 1024], BF16, tag="e1",
                                   name=f"e1_{p}_{t}")
                    nc.scalar.activation(e0[:], scA[:], AF.Exp)
                    if split[t]:
                        nc.vector.tensor_scalar(
                            e1[:].bitcast(I16), scB[:],
                            EXPS, EXPB, OP.mult, OP.add)
                    else:
                        nc.scalar.activation(e1[:], scB[:], AF.Exp)
                    if t + 1 < NT:
                        emit_scores(t + 1)
                    for hh in range(2):
                        h = 2 * p + hh
                        voff = (t % 8) * H * VW + h * VW
                        vt = v_sb[t // 8][:, voff:voff + VW]
                        for n in (0, 512):
                            eh = [e0, e1][n // 512]
                            nc.tensor.matmul(
                                acc[hh][:, n:n + 512], lhsT=vt,
                                rhs=eh[:, hh * 512:hh * 512 + 512],
                                start=(t == 0), stop=(t == NT - 1))
                if p == 1:
                    # out_ds[0] final since pair 0: start wo accumulation
                    # in the score banks (free after exp(31) reads them)
                    for mo in range(2):
                        po_t = ps.tile([128, SQ], F32,
                                       tag="psSA" if mo == 0 else "psSB",
                                       padded_shape=[128, 1024],
                                       name=f"po{mo}")
                        po.append(po_t)
                        for n in range(0, SQ, 512):
                            nc.tensor.matmul(
                                po_t[:, n:n + 512],
                                lhsT=wo_b[0][:, mo * 128:(mo + 1) * 128],
                                rhs=out_ds[0][:, n:n + 512],
                                start=True, stop=False)
                # drain the pair: normalize by the ones-column denominator
                asbs = []
                for hh in range(2):
                    asb = sb2.tile([VW, SQ], F32, tag="asb", name="asb")
                    if hh == 0:
                        nc.vector.tensor_copy(asb[:], acc[hh][:])
                    else:
                        nc.scalar.copy(asb[:], acc[hh][:])
                    asbs.append(asb)
                for hh in (1, 0):
                    nc.sync.dma_start(
                        out=den32[:, hh * 32:(hh + 1) * 32],
                        in_=asbs[hh][D:D + 1, :])
                with nc.allow_low_precision(reason="1/den broadcast in bf16"):
                    nc.vector.reciprocal(rc32[:], den32[:])
                # hh=1 first: its extra hsh->out_ds partition-shift DMA
                # overlaps hh=0's multiply
                for hh in (1, 0):
                    nc.sync.dma_start(out=rcd[hh][:],
                                      in_=rc32[:, hh * 32:(hh + 1) * 32])
                    rbc = sb2.tile([64, SQ], BF16, tag="rbc", name="rbc")
                    nc.sync.dma_start(out=rbc[:],
                                      in_=rcd[hh][0:1, :].broadcast_to(
                                          [64, SQ]))
                    if hh == 0:
                        nc.vector.tensor_mul(out_ds[p][0:64, :],
                                             asbs[hh][0:D, :], rbc[:])
                    else:
                        hsh = sb2.tile([64, SQ], BF16, tag="hsh", name="hsh")
                        nc.vector.tensor_mul(hsh[:], asbs[hh][0:D, :],
                                             rbc[:])
                        nc.sync.dma_start(out=out_ds[p][64:128, :],
                                          in_=hsh[:])

            # ---- stage 4: output projection + residual -----------------
            for mo in range(2):
                for n in range(0, SQ, 512):
                    nc.tensor.matmul(
                        po[mo][:, n:n + 512],
                        lhsT=wo_b[1][:, mo * 128:(mo + 1) * 128],
                        rhs=out_ds[1][:, n:n + 512],
                        start=False, stop=True)
                osb = sb2.tile([128, SQ], F32, tag="osb", name="osb")
                # xq has bo2 pre-added on the host; wocv restores the
                # groupnorm-fold v bias through wo; chunked so the out
                # DMA overlaps the second half's add
                for n in range(0, SQ, 512):
                    nc.vector.scalar_tensor_tensor(
                        osb[:, n:n + 512], po[mo][:, n:n + 512],
                        wocv[mo][:, 0:1], xq[mo][:, n:n + 512],
                        OP.add, OP.add)
                    for n2 in range(n, n + 512, 256):
                        nc.sync.dma_start(
                            out=out_d[mo * 128:(mo + 1) * 128, n2:n2 + 256],
                            in_=osb[:, n2:n2 + 256])

    _br.move_matmul_waits_to_ldweights(nc.m)
    _br.generate_event_semaphores(nc)
    return nc


# ---------------------------------------------------------------------------
# Host-side constants + input prep
# ---------------------------------------------------------------------------
def _consts():
    cidx = np.arange(C)
    pool = np.zeros((C, G), np.float32)
    pool[cidx, cidx // 8] = 1.0 / 8.0
    expand = np.zeros((G, C), np.float32)
    expand[cidx // 8, cidx] = 1.0
    return pool, expand


def make_in_maps(x, y, g1, b1, g2, b2, wq, bq, wk, bk, wv, bv, wo, bo):
    f = lambda a: np.ascontiguousarray(np.asarray(a, dtype=np.float32))
    bf = lambda a: np.ascontiguousarray(np.asarray(a).astype(ml_dtypes.bfloat16))
    x = f(x).reshape(2, C, S)
    y = f(y).reshape(2, C, S)
    xb16 = x.astype(ml_dtypes.bfloat16)
    pool, expand = _consts()
    bo2 = f(bo) + f(wo) @ f(bv)   # softmax-average commutes the v bias
    vecs = np.stack([f(bq) / 8.0, bo2, f(g1), f(b1), f(g2), f(b2)],
                    axis=1).astype(np.float32)
    base = {
        "wqT": bf(f(wq).T / 8.0),
        "wkT": bf(f(wk).T),
        "wvT": bf(f(wv).T),
        "woT": bf(f(wo).T),
        "vecs": np.ascontiguousarray(vecs),
        "poolm": pool, "expandm": expand,
    }
    in_maps = []
    for core in range(8):
        b, sq = core // 4, core % 4
        m = dict(base)
        m["x"] = np.ascontiguousarray(xb16[b])
        # roll y so the core's quarter sits at columns 0:SQ (q projection
        # reads yf[:, 0:SQ]); group stats are permutation-invariant
        m["y"] = np.ascontiguousarray(
            np.roll(y[b], -sq * SQ, axis=1).astype(ml_dtypes.bfloat16))
        m["xq"] = np.ascontiguousarray(
            x[b][:, sq * SQ:(sq + 1) * SQ] + bo2[:, None].astype(np.float32))
        in_maps.append(m)
    return in_maps


_NC_CACHE = None


def _get_nc():
    global _NC_CACHE
    if _NC_CACHE is None:
        _NC_CACHE = build_nc()
    return _NC_CACHE


def kernel(**inputs) -> np.ndarray:
    nc = _get_nc()
    in_maps = make_in_maps(**inputs)
    res = run_bass_kernel_spmd(nc, in_maps, core_ids=list(range(8)))
    out = np.empty((2, C, S), np.float32)
    for core in range(8):
        b, sq = core // 4, core % 4
        out[b][:, sq * SQ:(sq + 1) * SQ] = res.results[core]["out"]
    return out.reshape(2, C, 64, 64)


# revision 48
# speedup vs baseline: 1.0212x; 1.0212x over previous
"""MultiHeadAttnBlock TRN2 kernel (v2).

Full inputs -> shard across 8 NeuronCores -> full output.
Core i handles (batch b = i//4, spatial quarter sq = i%4): K/V over the
full spatial dim, Q over its quarter, 4-head attention for 1024 queries
x 4096 keys, wo projection, residual.

v2 changes vs the 235us baseline:
 - group-norm folded into the 1x1-conv weights: wkA/wvA = w * Ax[c],
   wqA = wq * Ay[c]; k-side biases vanish through softmax, the v-side
   bias is restored as wo@(wv@Bx) on the output, the q-side bias as a
   device matvec added at the q drain.  The [128,4096] normalize passes
   disappear.
 - scores for the two heads of a pair are emitted back-to-back into
   different PSUM banks with K=64 at partitions 0-63/64-127: the PE
   row-tiles them and streams both concurrently (~2x on scores).
 - one [128,2048] f32 score region per t-tile (banks 0-3), layout
   [h0n0|h1n0|h0n1|h1n1]; exp is issued per 2-bank half so the next
   tile's score matmuls ping-pong with the exp reads.
 - exp split across engines: ScalarE half0 (table exp), VectorE half1 on
   scheduled tiles via a bit-trick: i16 = rint(s*184.665+16250.49)
   reinterpreted as bf16 is exp(s) to ~3%; the denominator uses the same
   approximation so softmax cancels most of it.
 - rsqrt for the group stats via the 0x5f3759df bit trick + 2 Newton
   steps on VectorE: no Sqrt table set, single exp table load warmed at
   kernel start.
"""

import numpy as np
import ml_dtypes

import concourse.bass as bass
import concourse.mybir as mybir
import bass_rust as _br
from concourse.tile import TileContext
from concourse.bass_utils import run_bass_kernel_spmd

F32 = mybir.dt.float32
BF16 = mybir.dt.bfloat16
I16 = mybir.dt.int16
I32 = mybir.dt.int32
AF = mybir.ActivationFunctionType
OP = mybir.AluOpType

C = 256          # channels
S = 4096         # spatial (64*64)
SQ = 1024        # spatial quarter handled per core
H = 4            # heads
D = 64           # head dim
G = 32           # groups
EPS = 1e-6
NT = 32          # t tiles of 128 over S
VW = D + 2       # v' width per head (v | ones | pad)

EXPS = 184.66496523378732      # 128*log2(e)
EXPB = 16250.4931              # 128*127 - minimax centering
QUAKE = 1597463007.0           # 0x5f3759df


def _split_sched(p):
    """True -> VectorE computes the half1 exp of this t-tile."""
    if p == 0:
        return [t % 4 != 3 for t in range(NT)]
    return [t % 8 != 7 for t in range(NT)]


def build_nc():
    nc = bass.Bass("TRN2", target_bir_lowering=False, debug=False, num_devices=8)

    def din(name, shape, dt=F32):
        return nc.dram_tensor(name, shape, dt, kind="ExternalInput").ap()

    x_d = din("x", [C, S], BF16)    # full batch slice (stats + k/v)
    y_d = din("y", [C, S], BF16)    # full batch slice (stats + q quarter)
    xq_d = din("xq", [C, SQ])       # residual quarter + bo2, f32
    wqT_d = din("wqT", [C, C], BF16)   # wq.T / 8
    wkT_d = din("wkT", [C, C], BF16)
    wvT_d = din("wvT", [C, C], BF16)
    woT_d = din("woT", [C, C], BF16)
    # packed per-channel vectors: cols = (bq8, bo2, g1, b1, g2, b2)
    vecs_d = din("vecs", [C, 6])
    pool_d = din("poolm", [C, G])   # (c//8==g)/8
    exp_d = din("expandm", [G, C])  # (c//8==g)
    out_d = nc.dram_tensor("out", [C, SQ], F32, kind="ExternalOutput").ap()
    rcd = [nc.dram_tensor(f"rcd{i}", [1, SQ], BF16).ap() for i in range(2)]

    with TileContext(nc) as tc:
        with (
            tc.tile_pool(name="pers", bufs=1) as pers,
            tc.tile_pool(name="sb1", bufs=1) as sb1,
            tc.tile_pool(name="sb2", bufs=2) as sb2,
            tc.tile_pool(name="expp", bufs=3) as expp,
            tc.tile_pool(name="ps", bufs=1, space="PSUM") as ps,
        ):
            # ---- persistent tiles -------------------------------------
            xf = [pers.tile([128, S], BF16, tag=f"xf{m}", name=f"xf{m}")
                  for m in range(2)]
            yf = [pers.tile([128, S], BF16, tag=f"yf{m}", name=f"yf{m}")
                  for m in range(2)]
            xq = [pers.tile([128, SQ], F32, tag=f"xq{m}", name=f"xq{m}")
                  for m in range(2)]
            k_sb = [[pers.tile([128, 1024], BF16, tag=f"ksb{m}_{j}",
                               name=f"ksb{m}_{j}") for j in range(4)]
                    for m in range(2)]
            q_sb = [pers.tile([128, SQ], BF16, tag=f"qsb{m}", name=f"qsb{m}")
                    for m in range(2)]
            v_sb = [pers.tile([128, 8 * H * VW], BF16, tag=f"vsb{j}",
                              name=f"vsb{j}") for j in range(4)]
            out_ds = [pers.tile([128, SQ], BF16, tag=f"ods{m}", name=f"ods{m}")
                      for m in range(2)]
            wq_b = [pers.tile([128, C], BF16, tag=f"wqb{m}", name=f"wqb{m}")
                    for m in range(2)]
            wk_b = [pers.tile([128, C], BF16, tag=f"wkb{m}", name=f"wkb{m}")
                    for m in range(2)]
            wv_b = [pers.tile([128, C], BF16, tag=f"wvb{m}", name=f"wvb{m}")
                    for m in range(2)]
            wo_b = [pers.tile([128, C], BF16, tag=f"wob{m}", name=f"wob{m}")
                    for m in range(2)]
            wqA = [pers.tile([128, C], BF16, tag=f"wqA{m}", name=f"wqA{m}")
                   for m in range(2)]
            wkA = [pers.tile([128, C], BF16, tag=f"wkA{m}", name=f"wkA{m}")
                   for m in range(2)]
            wvA = [pers.tile([128, C], BF16, tag=f"wvA{m}", name=f"wvA{m}")
                   for m in range(2)]
            vecs = [pers.tile([128, 6], F32, tag=f"vecs{m}", name=f"vecs{m}")
                    for m in range(2)]
            _vc = {"bq8": 0, "bo2": 1, "g1": 2, "b1": 3, "g2": 4, "b2": 5}
            gb = {nm: [vecs[m][:, i:i + 1] for m in range(2)]
                  for nm, i in _vc.items()}
            cq8 = [pers.tile([128, 1], F32, tag=f"cq8{m}", name=f"cq8{m}")
                   for m in range(2)]
            cv_sb = [pers.tile([128, 1], F32, tag=f"cv{m}", name=f"cv{m}")
                     for m in range(2)]
            wocv = [pers.tile([128, 1], F32, tag=f"wocv{m}", name=f"wocv{m}")
                    for m in range(2)]
            den32 = pers.tile([32, 64], F32, tag="den32", name="den32")
            rc32 = pers.tile([32, 64], BF16, tag="rc32", name="rc32")
            warm = pers.tile([128, 2], F32, tag="warm", name="warm")
            ones_row = pers.tile([1, 64], BF16, tag="ones_row",
                                 name="ones_row")
            nc.gpsimd.memset(ones_row[:], 1.0)

            # ones column (64) + pad (65) of each v' head block
            for j in range(4):
                vview = v_sb[j][:].rearrange("p (t h e) -> p t h e", t=8, h=H)
                nc.gpsimd.memset(vview[:, :, :, D:VW], 1.0)

            # ---- stage 1: inputs + group-norm stats --------------------
            s6x = [sb1.tile([128, 24], F32, tag=f"s6x{m}", name=f"s6x{m}")
                   for m in range(2)]
            s6y = [sb1.tile([128, 24], F32, tag=f"s6y{m}", name=f"s6y{m}")
                   for m in range(2)]

            # tiny constants first: they gate the affine matmuls and must
            # not queue behind the big x/y transfers
            pool_sb = [sb1.tile([128, G], F32, tag=f"pl{m}", name=f"pl{m}")
                       for m in range(2)]
            expand_sb = sb1.tile([G, C], F32, tag="ex", name="ex")
            for m in range(2):
                nc.sync.dma_start(out=pool_sb[m][:],
                                  in_=pool_d[m * 128:(m + 1) * 128, :])
                nc.sync.dma_start(out=vecs[m][:],
                                  in_=vecs_d[m * 128:(m + 1) * 128, :])
            nc.sync.dma_start(out=expand_sb[:], in_=exp_d[:])
            # exp table warm-up: load the set while DMAs stream
            nc.scalar.activation(warm[:], vecs[0][:, 0:2], AF.Exp)

            # x first (k/v gate the pipeline): 8 chunks per half; stats on
            # alternating 512-chunks (half the DVE time, ~0.3% stat noise);
            # stat chunks DMA'd first so the stats finish early
            for ch in (0, 2, 4, 6, 1, 3, 5, 7):
                for m in range(2):
                    cs = slice(m * 128, (m + 1) * 128)
                    fs = slice(ch * 512, (ch + 1) * 512)
                    nc.sync.dma_start(out=xf[m][:, fs], in_=x_d[cs, fs])
                    if ch % 2 == 0:
                        c2 = ch // 2
                        nc.vector.bn_stats(s6x[m][:, c2 * 6:(c2 + 1) * 6],
                                           xf[m][:, fs])
            for m in range(2):
                nc.sync.dma_start(out=wk_b[m][:],
                                  in_=wkT_d[m * 128:(m + 1) * 128, :])
                nc.sync.dma_start(out=wv_b[m][:],
                                  in_=wvT_d[m * 128:(m + 1) * 128, :])
            for m in range(2):
                cs = slice(m * 128, (m + 1) * 128)
                for ch in (0, 2, 4, 6, 1, 3, 5, 7):
                    fs = slice(ch * 512, (ch + 1) * 512)
                    nc.sync.dma_start(out=yf[m][:, fs], in_=y_d[cs, fs])
            for m in range(2):
                cs = slice(m * 128, (m + 1) * 128)
                nc.sync.dma_start(out=wq_b[m][:], in_=wqT_d[cs, :])
                nc.sync.dma_start(out=wo_b[m][:], in_=woT_d[cs, :])
                for ch in range(2):
                    fs = slice(ch * 512, (ch + 1) * 512)
                    nc.sync.dma_start(out=xq[m][:, fs], in_=xq_d[cs, fs])

            def group_affine(s6, gamma, beta, tagp):
                """per-channel A, B [128,1] x2 from bn_stats chunks; rsqrt
                via the 0x5f3759df bit trick + 2 Newton steps (DVE only)"""
                ve = nc.vector
                stats_c = []
                for m in range(2):
                    mv = sb1.tile([128, 2], F32, tag=f"mv{tagp}{m}",
                                  name=f"mv{tagp}{m}")
                    ve.bn_aggr(mv[:], s6[m][:])
                    st = sb1.tile([128, 2], F32, tag=f"st{tagp}{m}",
                                  name=f"st{tagp}{m}")
                    ve.tensor_copy(st[:, 0:1], mv[:, 0:1])
                    msq = sb1.tile([128, 1], F32, tag=f"msq{tagp}{m}",
                                   name=f"msq{tagp}{m}")
                    ve.tensor_mul(msq[:], mv[:, 0:1], mv[:, 0:1])
                    ve.tensor_add(st[:, 1:2], mv[:, 1:2], msq[:])
                    stats_c.append(st)
                gp = ps.tile([G, 2], F32, tag="psD", padded_shape=[128, 1024],
                             name=f"gp{tagp}")
                for m in range(2):
                    nc.tensor.matmul(gp[:], lhsT=pool_sb[m][:],
                                     rhs=stats_c[m][:],
                                     start=(m == 0), stop=(m == 1))
                gs = sb1.tile([G, 2], F32, tag=f"gs{tagp}", name=f"gs{tagp}")
                nc.vector.tensor_copy(gs[:], gp[:])
                musq = sb1.tile([G, 1], F32, tag=f"gmusq{tagp}",
                                name=f"gmusq{tagp}")
                ve.tensor_mul(musq[:], gs[:, 0:1], gs[:, 0:1])
                veps = sb1.tile([G, 1], F32, tag=f"veps{tagp}",
                                name=f"veps{tagp}")
                ve.tensor_sub(veps[:], gs[:, 1:2], musq[:])
                ve.tensor_scalar_add(veps[:], veps[:], EPS)
                # quake rsqrt seed: float(bits) -> bits' = Q - bits/2
                fb = sb1.tile([G, 1], F32, tag=f"fb{tagp}", name=f"fb{tagp}")
                ve.tensor_copy(fb[:], veps[:].bitcast(I32))
                gi = sb1.tile([G, 1], I32, tag=f"gi{tagp}", name=f"gi{tagp}")
                ve.tensor_scalar(gi[:], fb[:], -0.5, QUAKE, OP.mult, OP.add)
                r = gi[:].bitcast(F32)
                # 2 Newton steps: r *= 1.5 - 0.5*veps*r^2
                y2 = sb1.tile([G, 1], F32, tag=f"gy2{tagp}",
                              name=f"gy2{tagp}")
                rr = sb1.tile([G, 1], F32, tag=f"grr{tagp}",
                              name=f"grr{tagp}")
                ve.tensor_mul(y2[:], r, r)
                ve.tensor_mul(y2[:], veps[:], y2[:])
                ve.tensor_scalar(y2[:], y2[:], -0.5, 1.5, OP.mult, OP.add)
                ve.tensor_mul(rr[:], r, y2[:])
                ve.tensor_mul(y2[:], rr[:], rr[:])
                ve.tensor_mul(y2[:], veps[:], y2[:])
                ve.tensor_scalar(y2[:], y2[:], -0.5, 1.5, OP.mult, OP.add)
                gs2 = sb1.tile([G, 2], F32, tag=f"gs2{tagp}",
                               name=f"gs2{tagp}")
                ve.tensor_mul(gs2[:, 0:1], rr[:], y2[:])
                ve.tensor_copy(gs2[:, 1:2], gs[:, 0:1])
                A, B = [], []
                for m in range(2):
                    pc = ps.tile([128, 2], F32, tag="psD",
                                 padded_shape=[128, 1024],
                                 name=f"pc{tagp}{m}")
                    nc.tensor.matmul(
                        pc[:], lhsT=expand_sb[:, m * 128:(m + 1) * 128],
                        rhs=gs2[:], start=True, stop=True)
                    a = sb1.tile([128, 1], F32, tag=f"A{tagp}{m}",
                                 name=f"A{tagp}{m}")
                    nc.vector.tensor_mul(a[:], pc[:, 0:1], gamma[m])
                    bmid = sb1.tile([128, 1], F32, tag=f"Bm{tagp}{m}",
                                    name=f"Bm{tagp}{m}")
                    nc.vector.tensor_mul(bmid[:], pc[:, 1:2], a[:])
                    b_ = sb1.tile([128, 1], F32, tag=f"B{tagp}{m}",
                                  name=f"B{tagp}{m}")
                    ve.tensor_sub(b_[:], beta[m], bmid[:])
                    A.append(a)
                    B.append(b_)
                return A, B

            Ax, Bx = group_affine(s6x, gb["g1"], gb["b1"], "x")
            for m in range(2):
                nc.vector.tensor_scalar(wkA[m][:], wk_b[m][:],
                                        Ax[m][:, 0:1], None, OP.mult)
                nc.vector.tensor_scalar(wvA[m][:], wv_b[m][:],
                                        Ax[m][:, 0:1], None, OP.mult)
            # cv = wv @ Bx (v bias from the folded norm; bv is host-folded)
            Bx16 = [sb1.tile([128, 1], BF16, tag=f"Bx16{m}", name=f"Bx16{m}")
                    for m in range(2)]
            cv16 = [sb1.tile([128, 1], BF16, tag=f"cv16{m}", name=f"cv16{m}")
                    for m in range(2)]
            for m in range(2):
                nc.vector.tensor_copy(Bx16[m][:], Bx[m][:])
            for m in range(2):
                pcv = ps.tile([128, 1], F32, tag="psC",
                              padded_shape=[128, 1024], name=f"pcv{m}")
                for kk in range(2):
                    nc.tensor.matmul(pcv[:],
                                     lhsT=wv_b[kk][:, m * 128:(m + 1) * 128],
                                     rhs=Bx16[kk][:, 0:1],
                                     start=(kk == 0), stop=(kk == 1))
                nc.vector.tensor_copy(cv16[m][:], pcv[:])

            # y stats AFTER the x affine chain: emission order is DVE
            # execution order, and affine-x gates the k/v projections
            for m in range(2):
                for c2 in range(4):
                    fs = slice(c2 * 1024, c2 * 1024 + 512)
                    nc.vector.bn_stats(s6y[m][:, c2 * 6:(c2 + 1) * 6],
                                       yf[m][:, fs])

            # ---- stage 2: k projection (no bias: softmax-invariant) ----
            for m in range(2):
                for n in range(0, S, 512):
                    pk = ps.tile([128, 512], F32,
                                 tag="psC" if (n // 512) % 2 == 0 else "psD",
                                 padded_shape=[128, 1024], name=f"pk{m}_{n}")
                    for kk in range(2):
                        nc.tensor.matmul(
                            pk[:],
                            lhsT=wkA[kk][:, m * 128:(m + 1) * 128],
                            rhs=xf[kk][:, n:n + 512],
                            start=(kk == 0), stop=(kk == 1))
                    kdst = k_sb[m][n // 1024][:, n % 1024:n % 1024 + 512]
                    nc.scalar.copy(kdst, pk[:])

            # ---- stage 2c: y affine, q projection ----------------------
            Ay, By = group_affine(s6y, gb["g2"], gb["b2"], "y")
            for m in range(2):
                nc.vector.tensor_scalar(wqA[m][:], wq_b[m][:],
                                        Ay[m][:, 0:1], None, OP.mult)
            # cq8 = (wq/8) @ By + bq/8
            By16 = [sb1.tile([128, 1], BF16, tag=f"By16{m}", name=f"By16{m}")
                    for m in range(2)]
            for m in range(2):
                nc.vector.tensor_copy(By16[m][:], By[m][:])
            for m in range(2):
                pcq = ps.tile([128, 1], F32, tag="psD",
                              padded_shape=[128, 1024], name=f"pcq{m}")
                for kk in range(2):
                    nc.tensor.matmul(pcq[:],
                                     lhsT=wq_b[kk][:, m * 128:(m + 1) * 128],
                                     rhs=By16[kk][:, 0:1],
                                     start=(kk == 0), stop=(kk == 1))
                nc.vector.tensor_scalar(cq8[m][:], pcq[:], gb["bq8"][m],
                                        None, OP.add)
            # wocv = wo @ cv (added at the output drain)
            for m in range(2):
                pwo = ps.tile([128, 1], F32, tag="psC",
                              padded_shape=[128, 1024], name=f"pwo{m}")
                for kk in range(2):
                    nc.tensor.matmul(pwo[:],
                                     lhsT=wo_b[kk][:, m * 128:(m + 1) * 128],
                                     rhs=cv16[kk][:, 0:1],
                                     start=(kk == 0), stop=(kk == 1))
                nc.vector.tensor_copy(wocv[m][:], pwo[:])
            # q projection over the quarter (host rolls y so it is cols 0:SQ)
            for m in range(2):
                pq = ps.tile([128, SQ], F32,
                             tag="psSA" if m == 0 else "psSB",
                             padded_shape=[128, 1024], name=f"pq{m}")
                for n in range(0, SQ, 512):
                    for kk in range(2):
                        nc.tensor.matmul(
                            pq[:, n:n + 512],
                            lhsT=wqA[kk][:, m * 128:(m + 1) * 128],
                            rhs=yf[kk][:, n:n + 512],
                            start=(kk == 0), stop=(kk == 1))
                nc.vector.tensor_scalar(q_sb[m][:], pq[:], cq8[m][:, 0:1],
                                        None, OP.add)

            # ---- stage 2b: v projection (all 32 t tiles) ---------------
            for t in range(NT):
                pv = ps.tile([128, C], F32,
                             tag="psC" if t % 2 == 0 else "psD",
                             padded_shape=[128, 1024], name=f"pv{t}")
                tsl = slice(t * 128, (t + 1) * 128)
                for kk in range(2):
                    nc.tensor.matmul(pv[:], lhsT=xf[kk][:, tsl],
                                     rhs=wvA[kk][:],
                                     start=(kk == 0), stop=(kk == 1))
                pvv = pv[:].rearrange("p (h e) -> p h e", h=H)
                dst = v_sb[t // 8][:, (t % 8) * H * VW:(t % 8 + 1) * H * VW]
                dvv = dst.rearrange("p (h e) -> p h e", h=H)[:, :, 0:D]
                if t < 24:
                    nc.scalar.copy(dvv, pvv)
                else:
                    nc.vector.tensor_copy(dvv, pvv)

            # ---- stage 3: attention ------------------------------------
            po = []
            for p in range(2):
                # two separate score tiles: Tile serializes cross-engine
                # accesses to one PSUM tile, so the ACT-read half and the
                # DVE-read half must be distinct tiles to run concurrently
                scA = ps.tile([128, 1024], F32, tag="psSA", name=f"scA{p}")
                scB = ps.tile([128, 1024], F32, tag="psSB", name=f"scB{p}")
                acc = [ps.tile([VW, SQ], F32, tag=["psC", "psD"][hh],
                               padded_shape=[128, 1024],
                               name=f"acc{p}_{hh}") for hh in range(2)]
                split = _split_sched(p)

                def emit_scores(t, nsel=(0, 512)):
                    # scA = [h0n0 | h1n0], scB = [h0n1 | h1n1]; head pair
                    # back-to-back -> concurrent row-tiled matmuls
                    tsl = slice((t % 8) * 128, (t % 8 + 1) * 128)
                    kt = k_sb[p][t // 8]
                    for n in nsel:
                        sct = scA if n == 0 else scB
                        for hh in range(2):
                            lo = hh * 64
                            nc.tensor.matmul(
                                sct[:, hh * 512:hh * 512 + 512],
                                lhsT=kt[lo:lo + 64, tsl],
                                rhs=q_sb[p][lo:lo + 64, n:n + 512],
                                start=True, stop=True)

                emit_scores(0)
                for t in range(NT):
                    # separate tiles per half so the ACT and DVE exps are
                    # independent writes and run concurrently
                    e0 = expp.tile([128, 1024], BF16, tag="e0",
                                   name=f"e0_{p}_{t}")
                    e1 = expp.tile([128, 1024], BF16, tag="e1",
                                   name=f"e1_{p}_{t}")
                    nc.scalar.activation(e0[:], scA[:], AF.Exp)
                    if split[t]:
                        nc.vector.tensor_scalar(
                            e1[:].bitcast(I16), scB[:],
                            EXPS, EXPB, OP.mult, OP.add)
                    else:
                        nc.scalar.activation(e1[:], scB[:], AF.Exp)
                    if t + 1 < NT:
                        emit_scores(t + 1)
                    for hh in range(2):
                        h = 2 * p + hh
                        voff = (t % 8) * H * VW + h * VW
                        vt = v_sb[t // 8][:, voff:voff + VW]
                        for n in (0, 512):
                            eh = [e0, e1][n // 512]
                            nc.tensor.matmul(
                                acc[hh][:, n:n + 512], lhsT=vt,
                                rhs=eh[:, hh * 512:hh * 512 + 512],
                                start=(t == 0), stop=(t == NT - 1))
                if p == 1:
                    # out_ds[0] final since pair 0: start wo accumulation
                    # in the score banks (free after exp(31) reads them)
                    for mo in range(2):
                        po_t = ps.tile([128, SQ], F32,
                                       tag="psSA" if mo == 0 else "psSB",
                                       padded_shape=[128, 1024],
                                       name=f"po{mo}")
                        po.append(po_t)
                        for n in range(0, SQ, 512):
                            nc.tensor.matmul(
                                po_t[:, n:n + 512],
                                lhsT=wo_b[0][:, mo * 128:(mo + 1) * 128],
                                rhs=out_ds[0][:, n:n + 512],
                                start=True, stop=False)
                # drain the pair: normalize by the ones-column denominator
                asbs = []
                for hh in range(2):
                    asb = sb2.tile([VW, SQ], F32, tag="asb", name="asb")
                    if hh == 0:
                        nc.vector.tensor_copy(asb[:], acc[hh][:])
                    else:
                        nc.scalar.copy(asb[:], acc[hh][:])
                    asbs.append(asb)
                for hh in range(2):
                    nc.sync.dma_start(
                        out=den32[:, hh * 32:(hh + 1) * 32],
                        in_=asbs[hh][D:D + 1, :])
                with nc.allow_low_precision(reason="1/den broadcast in bf16"):
                    nc.vector.reciprocal(rc32[:], den32[:])
                # hh=1 first: its extra hsh->out_ds partition-shift DMA
                # overlaps hh=0's multiply.  Pair 1 (the exposed tail)
                # broadcasts 1/den via a K=1 matmul into the freed acc
                # banks instead of the DRAM round trip: the PE is idle
                # there and two DMA dispatches drop off the chain.
                for hh in (1, 0):
                    if p == 1:
                        rcrow = sb2.tile([1, SQ], BF16, tag="rcrow",
                                         name="rcrow")
                        nc.sync.dma_start(out=rcrow[:],
                                          in_=rc32[:, hh * 32:(hh + 1) * 32])
                        rbc_ps = ps.tile([64, SQ], F32,
                                         tag="psC" if hh == 1 else "psD",
                                         padded_shape=[128, 1024],
                                         name=f"rbc{hh}")
                        for n in range(0, SQ, 512):
                            nc.tensor.matmul(rbc_ps[:, n:n + 512],
                                             lhsT=ones_row[:],
                                             rhs=rcrow[0:1, n:n + 512],
                                             start=True, stop=True)
                        rbc = rbc_ps
                    else:
                        nc.sync.dma_start(out=rcd[hh][:],
                                          in_=rc32[:, hh * 32:(hh + 1) * 32])
                        rbc = sb2.tile([64, SQ], BF16, tag="rbc", name="rbc")
                        nc.sync.dma_start(out=rbc[:],
                                          in_=rcd[hh][0:1, :].broadcast_to(
                                              [64, SQ]))
                    if hh == 0:
                        nc.vector.tensor_mul(out_ds[p][0:64, :],
                                             asbs[hh][0:D, :], rbc[:])
                    else:
                        hsh = sb2.tile([64, SQ], BF16, tag="hsh", name="hsh")
                        nc.vector.tensor_mul(hsh[:], asbs[hh][0:D, :],
                                             rbc[:])
                        nc.sync.dma_start(out=out_ds[p][64:128, :],
                                          in_=hsh[:])

            # ---- stage 4: output projection + residual -----------------
            for mo in range(2):
                for n in range(0, SQ, 512):
                    nc.tensor.matmul(
                        po[mo][:, n:n + 512],
                        lhsT=wo_b[1][:, mo * 128:(mo + 1) * 128],
                        rhs=out_ds[1][:, n:n + 512],
                        start=False, stop=True)
                osb = sb2.tile([128, SQ], F32, tag="osb", name="osb")
                # xq has bo2 pre-added on the host; wocv restores the
                # groupnorm-fold v bias through wo; chunked so the out
                # DMA overlaps the second half's add
                for n in range(0, SQ, 512):
                    nc.vector.scalar_tensor_tensor(
                        osb[:, n:n + 512], po[mo][:, n:n + 512],
                        wocv[mo][:, 0:1], xq[mo][:, n:n + 512],
                        OP.add, OP.add)
                    for n2 in range(n, n + 512, 256):
                        nc.sync.dma_start(
                            out=out_d[mo * 128:(mo + 1) * 128, n2:n2 + 256],
                            in_=osb[:, n2:n2 + 256])

    _br.move_matmul_waits_to_ldweights(nc.m)
    _br.generate_event_semaphores(nc)
    return nc


# ---------------------------------------------------------------------------
# Host-side constants + input prep
# ---------------------------------------------------------------------------
def _consts():
    cidx = np.arange(C)
    pool = np.zeros((C, G), np.float32)
    pool[cidx, cidx // 8] = 1.0 / 8.0
    expand = np.zeros((G, C), np.float32)
    expand[cidx // 8, cidx] = 1.0
    return pool, expand


def make_in_maps(x, y, g1, b1, g2, b2, wq, bq, wk, bk, wv, bv, wo, bo):
    f = lambda a: np.ascontiguousarray(np.asarray(a, dtype=np.float32))
    bf = lambda a: np.ascontiguousarray(np.asarray(a).astype(ml_dtypes.bfloat16))
    x = f(x).reshape(2, C, S)
    y = f(y).reshape(2, C, S)
    xb16 = x.astype(ml_dtypes.bfloat16)
    pool, expand = _consts()
    bo2 = f(bo) + f(wo) @ f(bv)   # softmax-average commutes the v bias
    vecs = np.stack([f(bq) / 8.0, bo2, f(g1), f(b1), f(g2), f(b2)],
                    axis=1).astype(np.float32)
    base = {
        "wqT": bf(f(wq).T / 8.0),
        "wkT": bf(f(wk).T),
        "wvT": bf(f(wv).T),
        "woT": bf(f(wo).T),
        "vecs": np.ascontiguousarray(vecs),
        "poolm": pool, "expandm": expand,
    }
    in_maps = []
    for core in range(8):
        b, sq = core // 4, core % 4
        m = dict(base)
        m["x"] = np.ascontiguousarray(xb16[b])
        # roll y so the core's quarter sits at columns 0:SQ (q projection
        # reads yf[:, 0:SQ]); group stats are permutation-invariant
        m["y"] = np.ascontiguousarray(
            np.roll(y[b], -sq * SQ, axis=1).astype(ml_dtypes.bfloat16))
        m["xq"] = np.ascontiguousarray(
            x[b][:, sq * SQ:(sq + 1) * SQ] + bo2[:, None].astype(np.float32))
        in_maps.append(m)
    return in_maps


_NC_CACHE = None


def _get_nc():
    global _NC_CACHE
    if _NC_CACHE is None:
        _NC_CACHE = build_nc()
    return _NC_CACHE


def kernel(**inputs) -> np.ndarray:
    nc = _get_nc()
    in_maps = make_in_maps(**inputs)
    res = run_bass_kernel_spmd(nc, in_maps, core_ids=list(range(8)))
    out = np.empty((2, C, S), np.float32)
    for core in range(8):
        b, sq = core // 4, core % 4
        out[b][:, sq * SQ:(sq + 1) * SQ] = res.results[core]["out"]
    return out.reshape(2, C, 64, 64)
